# revision 10
# baseline (speedup 1.0000x reference)
"""Trainium2 Bass kernel for int8 GEMM + fp32 bias (linear_a8_w8_bfp32_ofp32).

Computes out = (x_int8 @ weight_int8.T).astype(f32) + bias  for
x [8192, 4096] int8, weight [4096, 4096] int8, bias [4096] f32.

Strategy: column-parallel tensor parallelism over 8 NeuronCores — each core
gets all of x (replicated) and a 512-column slice of weight/bias, and
computes its [8192, 512] output slice.

The PE array has no int8 matmul mode (TRN2/cayman dropped UINT8), but
int8 values are exactly representable in bf16, bf16 x bf16 products
(<= 127*127) are exact, and PSUM accumulates in fp32 where every partial
sum of this data stays far below 2^24 — so a bf16 matmul reproduces the
int32-accumulated reference bit-exactly. fp8 can't beat this: an exact
int8 GEMM needs a >=3x nibble decomposition but DoubleRow only buys
~1.5-1.8x, so bf16 N=512 streaming (215.6 ns/MM) is the PE floor:
2048 MMs = 441.5 us/core.

v2 startup (vs the warmup-matmul baseline at 463.6us): no warmup — the
PE starts cold on real data as early as possible and warms while doing
useful work, which also removes the HAM re-throttle the baseline hit
when its warmup ran dry.  The first w k-tiles ride the otherwise-idle
sync+scalar HWDGE queues (parallel descriptor-gen with the gpsimd SWDGE
ring), w is staged as raw int8 and cast per-k-tile into 32 separate
[128,512] bf16 tiles, alternating DVE (even k) / scalar (odd k) so cast
throughput stays ahead of MM consumption.  x rides gpsimd casting DMAs
(int8->bf16 in the DMA) in 3 startup chunks for m-tile 0, then whole
tiles.  The last m-tile is split into two 256-wide PSUM chains so the
first half's bias-add + store hide behind the second half's matmuls.
"""

import numpy as np

import concourse.mybir as mybir
import concourse.tile as tile
from concourse import bacc
from concourse.bass_utils import run_bass_kernel_spmd

P = 128
N_CORES = 8

# Set by a test harness to capture timing/trace info; harmless defaults.
TRACE = False
TRACE_KWARGS = {}
LAST_RESULT = None


def build_program(MT, KT, NLOC, x_bufs=4, o_bufs=3, psum_bufs=4):
    """Bass/Tile program for one core: out[MT*128, NLOC] = xT.T @ wT + bias.

    DRAM layouts (host pre-arranged, all contiguous per SBUF partition):
      x_tiles   [MT, P, KT, P]  int8   x_tiles[mt, ki, kt, mi] = x[mt*P+mi, kt*P+ki]
      w_tiles   [P, KT, NLOC]   int8   w_tiles[ki, kt, n] = weight[n, kt*P+ki]
      bias_bcast[P, NLOC]       f32    bias replicated across partitions
      out_tiles [MT, P, NLOC]   f32    out_tiles[mt, mi, n] = out[mt*P+mi, n]
    """
    nc = bacc.Bacc()
    x_d = nc.declare_dram_parameter(
        "x_tiles", [MT, P, KT, P], mybir.dt.int8, isOutput=False
    )
    w_d = nc.declare_dram_parameter(
        "w_tiles", [P, KT, NLOC], mybir.dt.int8, isOutput=False
    )
    b_d = nc.declare_dram_parameter(
        "bias_bcast", [P, NLOC], mybir.dt.float32, isOutput=False
    )
    o_d = nc.declare_dram_parameter(
        "out_tiles", [MT, P, NLOC], mybir.dt.float32, isOutput=True
    )

    # w k-tiles 0-7 and 16-23 arrive raw int8 (HWDGE queues / SWDGE ring)
    # and are cast per-k-tile by DVE (even k, ~0.43us) / scalar (odd k,
    # ~0.71us); k-tiles 8-15 and 24-31 arrive as SWDGE *casting* DMAs into
    # contiguous bf16 tiles the matmuls read directly — this balances ring
    # write bytes against engine cast throughput so every k-tile beats its
    # consumption deadline during the cold-start ramp.
    # x m-tile 0: k0-3 raw over sync HWDGE + DVE cast (fastest path to the
    # first LDWEIGHTS), k4-31 as SWDGE casting DMAs.  x m-tile 1: k0-15
    # raw over sync HWDGE + DVE cast, k16-31 SWDGE casting.
    X0_CHUNKS = [(0, 4), (4, 16), (16, 32)]
    NH = NLOC // 2

    with tile.TileContext(nc) as tc:
        with (
            tc.tile_pool(name="wkpool", bufs=1) as wkpool,
            tc.tile_pool(name="wqpool", bufs=1) as wqpool,
            tc.tile_pool(name="cpool", bufs=1) as cpool,
            tc.tile_pool(name="x0pool", bufs=1) as x0pool,
            tc.tile_pool(name="xpool", bufs=x_bufs) as xpool,
            tc.tile_pool(name="opool", bufs=o_bufs) as opool,
            tc.tile_pool(name="otail", bufs=2) as otail,
            tc.tile_pool(name="psum", bufs=psum_bufs, space="PSUM") as psum_pool,
            tc.tile_pool(name="pst", bufs=2, space="PSUM") as pst_pool,
        ):
            # --- startup DMA emission -------------------------------------
            # sync(SP) + scalar(Act) HWDGE queues carry the first x/w
            # k-tiles (desc-to-data ~1.9us vs the SWDGE ring's ~3.3us and
            # parallel descriptor-gen); gpsimd (SWDGE, the only
            # casting-capable queue) carries the rest of x plus the later w
            # chunks.  Emission order per queue = execution order, so the
            # most-urgent chunks are emitted first.
            # sync HWDGE queue: x0 k0-3 raw, w k2-3, w k4-7, x1 k0-15 raw,
            # bias, then per-m-tile output stores.
            x0c0_raw = x0pool.tile(
                [P, X0_CHUNKS[0][1], P], mybir.dt.int8, tag="x0c0raw"
            )
            nc.sync.dma_start(out=x0c0_raw[:], in_=x_d[0, :, 0 : X0_CHUNKS[0][1], :])
            wq23 = wqpool.tile([P, 2, NLOC], mybir.dt.int8, tag="wq23")
            nc.sync.dma_start(out=wq23[:], in_=w_d[:, 2:4, :])
            wq47 = wqpool.tile([P, 4, NLOC], mybir.dt.int8, tag="wq47")
            nc.sync.dma_start(out=wq47[:], in_=w_d[:, 4:8, :])
            x1lo_raw = x0pool.tile([P, 16, P], mybir.dt.int8, tag="x1loraw")
            nc.sync.dma_start(out=x1lo_raw[:], in_=x_d[1, :, 0:16, :])
            b_sb = cpool.tile([P, NLOC], mybir.dt.float32)
            nc.sync.dma_start(out=b_sb[:], in_=b_d[:])

            # scalar HWDGE queue: w k0-1 raw, then its share of the casts.
            wq01 = wqpool.tile([P, 2, NLOC], mybir.dt.int8, tag="wq01")
            nc.scalar.dma_start(out=wq01[:], in_=w_d[:, 0:2, :])

            # gpsimd SWDGE ring, in deadline order: x0 k4-15 (cast),
            # w k8-15 (cast -> bf16 direct), x0 k16-31 (cast), w k16-23
            # (raw), w k24-27 + k28-31 (cast), x1 k16-31 (cast).
            x0_sb = [
                x0pool.tile(
                    [P, k1 - k0, P],
                    mybir.dt.bfloat16,
                    tag=f"x0c{ci}",
                    name=f"x0c{ci}",
                )
                for ci, (k0, k1) in enumerate(X0_CHUNKS)
            ]
            nc.gpsimd.dma_start(
                out=x0_sb[1][:],
                in_=x_d[0, :, X0_CHUNKS[1][0] : X0_CHUNKS[1][1], :],
            )
            wb815 = wkpool.tile([P, 8, NLOC], mybir.dt.bfloat16, tag="wb815")
            nc.gpsimd.dma_start(out=wb815[:], in_=w_d[:, 8:16, :])
            nc.gpsimd.dma_start(
                out=x0_sb[2][:],
                in_=x_d[0, :, X0_CHUNKS[2][0] : X0_CHUNKS[2][1], :],
            )
            wq1623 = wqpool.tile([P, 8, NLOC], mybir.dt.int8, tag="wq1623")
            nc.gpsimd.dma_start(out=wq1623[:], in_=w_d[:, 16:24, :])
            wb2427 = wkpool.tile([P, 4, NLOC], mybir.dt.bfloat16, tag="wb2427")
            nc.gpsimd.dma_start(out=wb2427[:], in_=w_d[:, 24:28, :])
            wb2831 = wkpool.tile([P, 4, NLOC], mybir.dt.bfloat16, tag="wb2831")
            nc.gpsimd.dma_start(out=wb2831[:], in_=w_d[:, 28:32, :])
            x1hi = x0pool.tile([P, 16, P], mybir.dt.bfloat16, tag="x1hi")
            nc.gpsimd.dma_start(out=x1hi[:], in_=x_d[1, :, 16:32, :])

            # --- engine casts (k0-7, k16-23 w; x0 k0-3; x1 k0-15) --------
            def w_stage_slice(k):
                if k < 2:
                    return wq01[:, k, :]
                if k < 4:
                    return wq23[:, k - 2, :]
                if k < 8:
                    return wq47[:, k - 4, :]
                return wq1623[:, k - 16, :]

            wk = {}
            for k in list(range(8)) + list(range(16, 24)):
                wk[k] = wkpool.tile(
                    [P, NLOC], mybir.dt.bfloat16, tag=f"wk{k}", name=f"wk{k}"
                )
            # DVE queue order = deadline order: x0c0, even k0-6, x1lo
            # halves, even k16-22, then m-loop bias adds.
            nc.vector.tensor_copy(x0_sb[0][:], x0c0_raw[:])
            for k in (0, 2, 4, 6):
                nc.vector.tensor_copy(wk[k][:], w_stage_slice(k))
            x1lo = x0pool.tile([P, 16, P], mybir.dt.bfloat16, tag="x1lo")
            nc.vector.tensor_copy(x1lo[:, 0:8, :], x1lo_raw[:, 0:8, :])
            nc.vector.tensor_copy(x1lo[:, 8:16, :], x1lo_raw[:, 8:16, :])
            for k in (16, 18, 20, 22):
                nc.vector.tensor_copy(wk[k][:], w_stage_slice(k))
            # scalar queue: odd k0-7, then odd k16-23.
            for k in (1, 3, 5, 7, 17, 19, 21, 23):
                nc.scalar.copy(wk[k][:], w_stage_slice(k))

            def w_slice(kt):
                if kt < 8 or 16 <= kt < 24:
                    return wk[kt][:]
                if kt < 16:
                    return wb815[:, kt - 8, :]
                if kt < 28:
                    return wb2427[:, kt - 24, :]
                return wb2831[:, kt - 28, :]

            # --- main m-tile loop -----------------------------------------
            def x_slice(mt, x_sb, kt):
                if mt == 0:
                    for ci, (k0, k1) in enumerate(X0_CHUNKS):
                        if k0 <= kt < k1:
                            return x0_sb[ci][:, kt - k0, :]
                    raise AssertionError(kt)
                if mt == 1:
                    if kt < 16:
                        return x1lo[:, kt, :]
                    return x1hi[:, kt - 16, :]
                return x_sb[:, kt, :]

            for mt in range(MT):
                if mt <= 1:
                    x_sb = None
                else:
                    x_sb = xpool.tile([P, KT, P], mybir.dt.bfloat16)
                    nc.gpsimd.dma_start(out=x_sb[:], in_=x_d[mt])
                if mt < MT - 1:
                    ps = psum_pool.tile([P, NLOC], mybir.dt.float32)
                    for kt in range(KT):
                        nc.tensor.matmul(
                            ps[:],
                            x_slice(mt, x_sb, kt),
                            w_slice(kt),
                            start=(kt == 0),
                            stop=(kt == KT - 1),
                        )
                    o_sb = opool.tile([P, NLOC], mybir.dt.float32)
                    nc.vector.tensor_add(o_sb[:], ps[:], b_sb[:])
                    nc.sync.dma_start(out=o_d[mt], in_=o_sb[:])
                else:
                    # last m-tile: two 256-wide chains so the first half's
                    # epilogue hides behind the second half's matmuls, and
                    # the final exposed tail is only a half-width epilogue.
                    for h in range(2):
                        ph = pst_pool.tile([P, NH], mybir.dt.float32, tag=f"pst{h}")
                        for kt in range(KT):
                            w_ap = w_slice(kt)
                            nc.tensor.matmul(
                                ph[:],
                                x_slice(mt, x_sb, kt),
                                w_ap[:, h * NH : (h + 1) * NH],
                                start=(kt == 0),
                                stop=(kt == KT - 1),
                            )
                        o_h = otail.tile([P, NH], mybir.dt.float32, tag=f"ot{h}")
                        nc.vector.tensor_add(
                            o_h[:], ph[:], b_sb[:, h * NH : (h + 1) * NH]
                        )
                        # parallel desc-gen: half 0 on sync, half 1 on scalar
                        eng = nc.sync if h == 0 else nc.scalar
                        eng.dma_start(
                            out=o_d[mt, :, h * NH : (h + 1) * NH], in_=o_h[:]
                        )
    nc.compile()
    return nc


def run(x, weight, fake_bias):
    global LAST_RESULT
    M, K = x.shape
    N = weight.shape[0]
    assert M % P == 0 and K % P == 0 and N % (N_CORES * P) == 0
    MT, KT, NLOC = M // P, K // P, N // N_CORES

    xb = np.asarray(x).astype(np.int8)
    x_tiles = np.ascontiguousarray(xb.reshape(MT, P, KT, P).transpose(0, 3, 2, 1))
    wb = np.asarray(weight).astype(np.int8)
    bias = np.asarray(fake_bias).astype(np.float32)

    in_maps = []
    for c in range(N_CORES):
        w_loc = wb[c * NLOC : (c + 1) * NLOC, :]  # [NLOC, K]
        w_tiles = np.ascontiguousarray(
            w_loc.T.reshape(KT, P, NLOC).transpose(1, 0, 2)
        )
        b_loc = np.ascontiguousarray(
            np.broadcast_to(bias[None, c * NLOC : (c + 1) * NLOC], (P, NLOC))
        )
        in_maps.append(
            {"x_tiles": x_tiles, "w_tiles": w_tiles, "bias_bcast": b_loc}
        )

    nc = build_program(MT, KT, NLOC)
    res = run_bass_kernel_spmd(
        nc, in_maps, list(range(N_CORES)), trace=TRACE, **TRACE_KWARGS
    )
    LAST_RESULT = res

    outs = [r["out_tiles"].reshape(M, NLOC) for r in res.results]
    return np.concatenate(outs, axis=1).astype(np.float32)


def kernel(x, weight, fake_bias):
    return run(x, weight, fake_bias)
